# revision 9
# baseline (speedup 1.0000x reference)
"""Trainium2 Bass kernel for ConvTemporalGraphical (gnn_message_passing).

Reference computation (fp32):
    y   = einsum('nctv,oc->notv', x, W) + b        # 1x1 conv channel mix
    out = einsum('nkctv,kvw->nctw', y.reshape(n,K,C,t,v), A)

Shapes: x [16,128,256,64] f32, A [3,64,64], W [384,128], b [384].

Strategy (8 NeuronCores, data-parallel over N, 2 samples per core):
  W-contraction first, producing the intermediate TRANSPOSED so no PE
  transposes are needed anywhere:
      stage 1:  yT[(t,v), (k,c)] = sum_ci x[ci,t,v] * Wt[ci,(k,c)]
                (lhsT = x in its natural layout, fp32r, FD=384)
      stage 2:  out[c, (t,w)]   += sum_{v} yT[(t,v),(k,c)] * A[k,v,w]
                (bf16, FD=128, PSUM-accumulated over k; MA_k block-diag
                 over the two t's of a pair to use all 128 partitions)

  The bias is folded into the stage-1 PSUM drain: with
      T[w,(k,v)] = A[k,v,w],   beta = T^+ @ bias2^T   (min-norm preimage)
  adding beta[(v),(k,c)] to yT before stage 2 reproduces
  bias2[c,w] = sum_{k,v} b[(k,c)] A[k,v,w] exactly (residual ~1e-6, beta
  scale ~0.07 so no bf16 amplification). The drain is then a
  TensorTensor add (PSUM f32 + beta f32 -> SBUF bf16), and the final
  output drain is a plain f32 copy (PSUM is not DMA- or GpSimd-
  accessible, so DVE/ACT must carry it).

  Measured pacing (this hw): FD=384 f32r b2b 195ns/MM, FD=128 bf16 b2b
  62ns/MM -> PE ~1525ns per 8-t group = ~98us/core. DVE/ACT carry only
  4 384-elem drain-adds per group (~1.1us/group split across both).

kernel(**inputs) shards on host, runs the SPMD program on cores 0-7, and
concatenates the per-core outputs.
"""

import numpy as np
import ml_dtypes

import concourse.bass as bass
import concourse.mybir as mybir
from concourse import bacc
from concourse.bass_utils import run_bass_kernel_spmd
from concourse.tile import TileContext

F32 = mybir.dt.float32
F32R = mybir.dt.float32r
BF16 = mybir.dt.bfloat16

N, C_IN, C_OUT, K, T, V = 16, 128, 128, 3, 256, 64
N_CORES = 8
N_PER_CORE = N // N_CORES  # 2
G = 8                      # t's per group
N_GROUPS = T // G          # 32 groups per sample


def build(reps: int = 1):
    nc = bacc.Bacc(
        "TRN2", target_bir_lowering=False, debug=False, num_devices=N_CORES
    )
    xs = nc.dram_tensor("xs", [N_PER_CORE, C_IN, T, V], F32, kind="ExternalInput")
    wt = nc.dram_tensor("wt", [C_IN, K * C_OUT], F32, kind="ExternalInput")
    mak = nc.dram_tensor("mak", [128, K, 128], BF16, kind="ExternalInput")
    beta = nc.dram_tensor("beta", [128, K * C_OUT], F32, kind="ExternalInput")
    out = nc.dram_tensor(
        "out", [N_PER_CORE, C_OUT, T, V], F32, kind="ExternalOutput"
    )

    with TileContext(nc) as tc:
        with (
            tc.tile_pool(name="const", bufs=1) as cpool,
            tc.tile_pool(name="xin", bufs=6) as xpool,
            tc.tile_pool(name="yt", bufs=3) as ytpool,
            tc.tile_pool(name="o", bufs=3) as opool,
            tc.tile_pool(name="ps_y", bufs=5, space="PSUM") as ps_y,
            tc.tile_pool(name="ps_o", bufs=3, space="PSUM") as ps_o,
        ):
            # consts on the gpsimd DMA queue so the sync queue's first x-tile
            # descriptor issues immediately
            wt_sb = cpool.tile([C_IN, K * C_OUT], F32R, tag="wt")
            nc.gpsimd.dma_start(out=wt_sb[:], in_=wt[:].bitcast(F32R))
            mak_sb = cpool.tile([128, K, 128], BF16, tag="mak")
            nc.gpsimd.dma_start(out=mak_sb[:], in_=mak[:])
            beta_sb = cpool.tile([128, K * C_OUT], F32, tag="beta")
            nc.gpsimd.dma_start(out=beta_sb[:], in_=beta[:])

            # Software-pipelined emission: stage 1 of group i runs while
            # stage 2 of group i-1 consumes yT drained during i's stage 1.
            for _ in range(reps):
                groups = [
                    (n, g) for n in range(N_PER_CORE) for g in range(N_GROUPS)
                ]
                st = {}

                def stage1(n, g):
                    x_sb = xpool.tile([C_IN, G * V], F32R, tag="x", name="x_sb")
                    t0 = g * G
                    nc.sync.dma_start(
                        out=x_sb[:],
                        in_=xs[n, :, t0 : t0 + G, :].bitcast(F32R),
                    )
                    yt_sb = ytpool.tile(
                        [128, 4, K * C_OUT], BF16, tag="yt", name="yt_sb"
                    )
                    for j in range(4):
                        yt_ps = ps_y.tile([128, 512], F32, tag="ytp")
                        nc.tensor.matmul(
                            yt_ps[:, 0 : K * C_OUT],
                            x_sb[:, j * 128 : (j + 1) * 128],
                            wt_sb[:],
                            start=True,
                            stop=True,
                        )
                        # drain-add: yT + beta -> bf16 (bias folded in)
                        nc.any.tensor_add(
                            out=yt_sb[:, j, :],
                            in0=yt_ps[:, 0 : K * C_OUT],
                            in1=beta_sb[:],
                        )
                    st[(n, g)] = yt_sb

                def stage2(n, g):
                    yt_sb = st.pop((n, g))
                    o_ps = ps_o.tile([C_OUT, 4, 2 * V], F32, tag="op")
                    for j in range(4):
                        for k in range(K):
                            nc.tensor.matmul(
                                o_ps[:, j, :],
                                yt_sb[:, j, k * 128 : (k + 1) * 128],
                                mak_sb[:, k, :],
                                start=(k == 0),
                                stop=(k == K - 1),
                                skip_group_check=True,
                            )
                    # plain f32 drain (bias already folded), then DMA out
                    o_sb = opool.tile([C_OUT, G * V], F32, tag="o", name="o_sb")
                    nc.any.tensor_copy(out=o_sb[:], in_=o_ps[:])
                    t0 = g * G
                    nc.gpsimd.dma_start(
                        out=out[n, :, t0 : t0 + G, :],
                        in_=o_sb[:],
                    )

                for i in range(len(groups) + 1):
                    if i < len(groups):
                        stage1(*groups[i])
                    if i >= 1:
                        stage2(*groups[i - 1])

    nc.compile()
    return nc


def prep_weights(A, W, b):
    A = np.asarray(A, np.float32)
    W = np.asarray(W, np.float32)
    b = np.asarray(b, np.float32)
    # wt[ci, (k,c)]
    wt = np.ascontiguousarray(
        W.reshape(K, C_OUT, C_IN).transpose(2, 0, 1).reshape(C_IN, K * C_OUT)
    )
    # mak[(h,v), k, (h',w)] = A[k,v,w] * delta_{h,h'}
    m = np.zeros((2, V, K, 2, V), np.float32)
    for h in range(2):
        m[h, :, :, h, :] = A.transpose(1, 0, 2)
    mak = m.reshape(128, K, 128).astype(ml_dtypes.bfloat16)
    # bias fold: beta[(h,v), (k,c)] with sum_{k,v} beta A = bias2
    bias2 = np.einsum("kc,kw->cw", b.reshape(K, C_OUT), A.sum(axis=1))
    Tm = A.transpose(2, 0, 1).reshape(V, K * V)  # [w, (k,v)]
    bp, *_ = np.linalg.lstsq(Tm, bias2.T, rcond=None)  # [(k,v), c]
    bvkc = bp.reshape(K, V, C_OUT).transpose(1, 0, 2)  # [v, k, c]
    beta = np.broadcast_to(
        bvkc.reshape(1, V, K * C_OUT), (2, V, K * C_OUT)
    ).reshape(128, K * C_OUT)
    beta = np.ascontiguousarray(beta).astype(np.float32)
    return wt, mak, beta


_NC_CACHE = {}


def get_nc(reps: int = 1):
    if reps not in _NC_CACHE:
        _NC_CACHE[reps] = build(reps)
    return _NC_CACHE[reps]


def make_in_maps(x, A, W, b):
    x = np.asarray(x, np.float32)
    wt, mak, beta = prep_weights(A, W, b)
    return [
        {
            "xs": np.ascontiguousarray(x[i * N_PER_CORE : (i + 1) * N_PER_CORE]),
            "wt": wt,
            "mak": mak,
            "beta": beta,
        }
        for i in range(N_CORES)
    ]


def run(x, A, W, b, reps: int = 1):
    nc = get_nc(reps)
    in_maps = make_in_maps(x, A, W, b)
    res = run_bass_kernel_spmd(nc, in_maps, list(range(N_CORES)))
    return np.concatenate(
        [np.asarray(res.results[i]["out"]) for i in range(N_CORES)], axis=0
    )


def kernel(x, A, W, b):
    return run(x, A, W, b, reps=1)
